# revision 39
# baseline (speedup 1.0000x reference)
"""AttentionBlock (B=4, C=256, H=W=64) on 8 Trainium2 NeuronCores.

Sharding: data-parallel over (batch, query-half): core i handles batch i//2,
query pixels [half*2048, (half+1)*2048), half = i%2. GroupNorm stats are
computed per batch element (duplicated across the pair, cheap); the O(N^2)
attention work is fully sharded 8 ways. No collectives.

v5: all large matmuls are fp8e4 DoubleRow (K=256 per 512-row instruction).
k and v are never materialized -- by associativity the attention runs on
raw fp8 x from both ends:
    S  = k^T q       = x^T (Wk^T q)           (qk made once per chunk)
    O  = v E         = Wv (x E) + bv R        (xE accumulated in PSUM)
so the inner loop is just S(pair) -> exp -> xE/R accumulate, identical for
all 64 pairs, with zero per-pair weight/bias traffic. The GN channel scale
rides the per-partition qk/q drains, bv/bq fold into per-partition drain
biases, and the k-side bias is dropped outright (a per-query logit shift is
softmax-invariant). exp runs on ACT in [128,1024] chunks into fp8 with a -3
logit shift (e4m3 range), software-pipelined one S-pair ahead of the xE/R
consumers. 1/R uses reciprocal_approx_fast on the PE-broadcast R block.
Host ships x twice in fp8 ([cin, pix] and [pix, cin], query half first),
the residual base x+rbias in f32, and weights in bf16 (GN fold + 16x fp8
cast on device; Wk^T additionally raw fp8).
"""

import numpy as np

B, C, HW = 4, 256, 4096
NH = 2048            # query pixels per core
G, CPG = 32, 8       # groups, channels per group
EPS = 1e-5
MB = HW // 128       # 32 key blocks
NP = MB // 2         # 16 key-block pairs
SW = 16.0            # fp8 weight pre-scale

_cache = {}


def build_nc():
    """Build (and cache) the Bass module."""
    if "nc" in _cache:
        return _cache["nc"]
    import concourse.tile as tile
    from concourse import bacc, mybir

    f32 = mybir.dt.float32
    bf16 = mybir.dt.bfloat16
    f8 = mybir.dt.float8e4
    AF = mybir.ActivationFunctionType
    OP = mybir.AluOpType
    PM = mybir.MatmulPerfMode

    nc = bacc.Bacc("TRN2", target_bir_lowering=False, debug=False,
                   enable_asserts=False, num_devices=8)

    # ---- DRAM I/O (host preps everything into device layout) ----
    d_xf = nc.dram_tensor("xf", [128, 2, HW], f8, kind="ExternalInput")
    d_xt = nc.dram_tensor("xt", [128, MB, C], f8, kind="ExternalInput")
    d_xo = nc.dram_tensor("xo", [128, 2, NH], f32, kind="ExternalInput")
    d_w = nc.dram_tensor("w", [128, 2, 4 * C], bf16, kind="ExternalInput")
    d_wkt = nc.dram_tensor("wkt", [128, 2, C], f8, kind="ExternalInput")
    d_sb = nc.dram_tensor("sb", [128, 2, 3], f32, kind="ExternalInput")
    d_ag = nc.dram_tensor("ag", [128, 2, G], f32, kind="ExternalInput")
    d_bg = nc.dram_tensor("bg", [G, 2, 128], f32, kind="ExternalInput")
    d_out = nc.dram_tensor("out", [128, 2, NH], f32, kind="ExternalOutput")

    with tile.TileContext(nc) as tc:
        with (
            tc.tile_pool(name="big", bufs=1) as big,
            tc.tile_pool(name="cst", bufs=1) as cst,
            tc.tile_pool(name="wrk", bufs=2) as wrk,
            tc.tile_pool(name="epool", bufs=4) as epool,
            tc.tile_pool(name="gnp", bufs=1) as gnp,
            tc.tile_pool(name="ps_s", bufs=2, space="PSUM") as ps_s,
            tc.tile_pool(name="ps_o", bufs=1, space="PSUM") as ps_o,
            tc.tile_pool(name="ps_r", bufs=1, space="PSUM") as ps_r,
            tc.tile_pool(name="ps_x", bufs=1, space="PSUM") as ps_x,
        ):
            # ---- input DMAs: x first (gates GN stats); big descriptors,
            # few triggers; params on scalar after x ----
            # scalar ring: weights first (their consumers start ~20us; the
            # scalar-half x quarters still outrun the DVE-serial bn_stats)
            wall = cst.tile([128, 2, 4 * C], bf16, tag="wall")
            nc.scalar.dma_start(out=wall, in_=d_w.ap())
            wkt8 = cst.tile([128, 2, C], f8, tag="wkt8")
            nc.scalar.dma_start(out=wkt8, in_=d_wkt.ap())
            xf8 = big.tile([128, 2, HW], f8, tag="xf8")
            bstat = gnp.tile([128, 2, 8, 6], f32, tag="bstat")
            for ci in range(2):
                for h in range(2):
                    for q in range(2):
                        sl = slice(h * NH + q * 1024, h * NH + (q + 1) * 1024)
                        eng = nc.sync if h == 0 else nc.scalar
                        eng.dma_start(out=xf8[:, ci, sl],
                                      in_=d_xf.ap()[:, ci, sl])
            xt8 = big.tile([128, MB, C], f8, tag="xt8")
            for h in range(2):
                sl = slice(h * MB // 2, (h + 1) * MB // 2)
                eng = nc.sync if h == 0 else nc.scalar
                eng.dma_start(out=xt8[:, sl, :], in_=d_xt.ap()[:, sl, :])
            for ci in range(2):
                for j in range(8):
                    nc.vector.bn_stats(
                        out=bstat[:, ci, j, :],
                        in_=xf8[:, ci, j * 512:(j + 1) * 512])
            smalls = cst.tile([128, 2, 3], f32, tag="smalls")
            nc.sync.dma_start(out=smalls, in_=d_sb.ap())
            qb = smalls[:, :, 0:1]
            gb = smalls[:, :, 2:3]
            ag = cst.tile([128, 2, G], f32, tag="ag")
            nc.sync.dma_start(out=ag, in_=d_ag.ap())
            bg = cst.tile([G, 2, 128], f32, tag="bg")
            nc.sync.dma_start(out=bg, in_=d_bg.ap())
            xo = big.tile([128, 2, NH], f32, tag="xo")
            for ci in range(2):
                nc.sync.dma_start(out=xo[:, ci, :], in_=d_xo.ap()[:, ci, :])

            epst = cst.tile([G, 1], f32, tag="epst")
            nc.vector.memset(epst, EPS)
            negc = cst.tile([128, 1], f32, tag="negc")  # softmax logit shift
            nc.vector.memset(negc, -3.0)
            # R lhsT (DoubleRow, M=128: R lands pre-broadcast on all rows)
            ones8 = cst.tile([128, 2, 128], f8, tag="ones8")
            nc.vector.memset(ones8, 1.0)
            # warm the PE through the DMA/stats window: ~13us of back-to-back
            # dummy matmuls ramps the clock out of the cold p-state before the
            # GN matmuls and the q/qk/S prologue chain run
            dums = cst.tile([128, 2, 512], f8, tag="dums")
            nc.vector.memset(dums, 0.5)
            for w in range(30):
                pw = ps_o.tile([128, 2, 512], f32, tag="o", name=f"warm{w}")
                nc.tensor.matmul(pw[:, 0, :], lhsT=ones8, rhs=dums,
                                 start=True, stop=True,
                                 perf_mode=PM.DoubleRow)

            # ---- GroupNorm stats aggregation ----
            stats2 = gnp.tile([128, 2, 2], f32, tag="stats2")  # (mean, E[x^2])
            tmp1 = gnp.tile([128, 1], f32, tag="tmp1")
            for ci in range(2):
                nc.vector.bn_aggr(out=stats2[:, ci, :], in_=bstat[:, ci, :, :])
                nc.vector.tensor_tensor(
                    out=tmp1, in0=stats2[:, ci, 0:1], in1=stats2[:, ci, 0:1],
                    op=OP.mult)
                nc.vector.tensor_tensor(
                    out=stats2[:, ci, 1:2], in0=stats2[:, ci, 1:2], in1=tmp1,
                    op=OP.add)
            # group sums across partitions: [G, 2] = sum_ci ag[ci]^T stats2[ci]
            pg = ps_x.tile([G, 2], f32, tag="x")
            for ci in range(2):
                nc.tensor.matmul(pg, lhsT=ag[:, ci, :], rhs=stats2[:, ci, :],
                                 start=(ci == 0), stop=(ci == 1))
            # ag carries 1/CPG so pg is directly (mean_g, E[x^2]_g)
            pgs = gnp.tile([G, 2], f32, tag="pgs")
            nc.vector.tensor_copy(out=pgs, in_=pg)
            gst = gnp.tile([G, 4], f32, tag="gst")  # mean^2, var, sd, -
            nc.vector.tensor_tensor(out=gst[:, 0:1], in0=pgs[:, 0:1],
                                    in1=pgs[:, 0:1], op=OP.mult)
            nc.vector.tensor_tensor(out=gst[:, 1:2], in0=pgs[:, 1:2],
                                    in1=gst[:, 0:1], op=OP.subtract)
            gfin = gnp.tile([G, 2], f32, tag="gfin")  # (rstd_g, mean_g*rstd_g)
            nc.scalar.activation(out=gst[:, 2:3], in_=gst[:, 1:2],
                                 func=AF.Sqrt, bias=epst)
            nc.vector.reciprocal(out=gfin[:, 0:1], in_=gst[:, 2:3])
            nc.vector.tensor_tensor(out=gfin[:, 1:2], in0=pgs[:, 0:1],
                                    in1=gfin[:, 0:1], op=OP.mult)
            # bg carries gn_w, so pbc = (scale_c, mean_c*scale_c);
            # bias_c = gn_b - mean_c*scale_c
            scbc = gnp.tile([128, 2, 2], f32, tag="scbc")
            for ci in range(2):
                pbc = ps_x.tile([128, 2], f32, tag="x")
                nc.tensor.matmul(pbc, lhsT=bg[:, ci, :], rhs=gfin,
                                 start=True, stop=True)
                nc.vector.tensor_copy(out=scbc[:, ci, 0:1], in_=pbc[:, 0:1])
                nc.vector.tensor_tensor(out=scbc[:, ci, 1:2], in0=gb[:, ci, :],
                                        in1=pbc[:, 1:2], op=OP.subtract)

            # ---- fp8 weights (gate q/qk production):
            # W8 = W * scale_c * 16 (q,v), W * 16 (proj)
            w8 = cst.tile([128, 2, 4 * C], f8, tag="w8")
            for ci in range(2):
                nc.vector.tensor_scalar(
                    out=w8[:, ci, 0:3 * C], in0=wall[:, ci, 0:3 * C],
                    scalar1=scbc[:, ci, 0:1], scalar2=SW,
                    op0=OP.mult, op1=OP.mult)
                nc.vector.tensor_scalar(
                    out=w8[:, ci, 3 * C:4 * C], in0=wall[:, ci, 3 * C:4 * C],
                    scalar1=SW, scalar2=None, op0=OP.mult)

            qt = big.tile([128, 2, NH], f8, tag="qt")
            qk = big.tile([128, 2, NH], f8, tag="qk")

            def qt_unit(j):  # q for 512 queries: Wq' x (bias rides in qk)
                sl = slice(j * 512, (j + 1) * 512)
                pq = ps_s.tile([128, 2, 512], f32, tag="s", name=f"pq{j}")
                for cb in range(2):
                    nc.tensor.matmul(
                        pq[:, cb, :],
                        lhsT=w8[:, :, cb * 128:(cb + 1) * 128],
                        rhs=xf8[:, :, sl], start=True, stop=True,
                        perf_mode=PM.DoubleRow)
                nc.vector.tensor_scalar(
                    out=qt[:, :, sl], in0=pq, scalar1=1.0 / SW, scalar2=None,
                    op0=OP.mult)

            qt_unit(0)

            # ---- per-partition drain biases from the GN fold (bias_c on
            # unscaled bf16 W). The k-side bias shifts every logit of a query
            # equally -- softmax-invariant -- so it is dropped.
            bcr = gnp.tile([128, 2, 2], bf16, tag="bcr")  # bias_c, 2 copies
            for ci in range(2):
                nc.vector.tensor_copy(out=bcr[:, ci, 0:1],
                                      in_=scbc[:, ci, 1:2])
                nc.vector.tensor_copy(out=bcr[:, ci, 1:2],
                                      in_=scbc[:, ci, 1:2])
            # bias2[:, cb, 0] = qb + Wq @ bias_c ; bias2[:, cb, 1] = Wv @ bias_c
            bias2 = gnp.tile([128, 2, 2], f32, tag="bias2")
            for wi, woff in ((0, 0), (1, 2 * C)):
                for cb in range(2):
                    pbias = ps_x.tile([128, 2], f32, tag="x")
                    for ci in range(2):
                        nc.tensor.matmul(
                            pbias,
                            lhsT=wall[:, ci, woff + cb * 128:
                                      woff + (cb + 1) * 128],
                            rhs=bcr[:, ci, :], start=(ci == 0), stop=(ci == 1))
                    if wi == 0:
                        nc.vector.tensor_tensor(
                            out=bias2[:, cb, 0:1], in0=pbias[:, 0:1],
                            in1=qb[:, cb, :], op=OP.add)
                    else:
                        nc.vector.tensor_copy(out=bias2[:, cb, 1:2],
                                              in_=pbias[:, 0:1])
            # fold the q bias into the qk drain instead of the q drain:
            # qk = s/16 * Wk16^T (q0 + bq) = (pk + qkb) * sc16,
            # qkb = Wk16^T bq16 / 16. qt then drains biasless in one op.
            bqr8 = gnp.tile([128, 2, 2], f8, tag="bqr8")
            for cb in range(2):
                for k in range(2):
                    nc.vector.tensor_scalar(
                        out=bqr8[:, cb, k:k + 1], in0=bias2[:, cb, 0:1],
                        scalar1=SW, scalar2=None, op0=OP.mult)
            qkbias = gnp.tile([128, 2, 1], f32, tag="qkbias")
            sc16 = gnp.tile([128, 2, 1], f32, tag="sc16")
            for ci in range(2):
                pqkb = ps_x.tile([128, 2], f32, tag="x")
                nc.tensor.matmul(
                    pqkb, lhsT=wkt8[:, :, ci * 128:(ci + 1) * 128],
                    rhs=bqr8, start=True, stop=True, perf_mode=PM.DoubleRow)
                nc.vector.tensor_scalar(
                    out=qkbias[:, ci, :], in0=pqkb[:, 0:1],
                    scalar1=1.0 / SW, scalar2=None, op0=OP.mult)
                nc.vector.tensor_scalar(
                    out=sc16[:, ci, :], in0=scbc[:, ci, 0:1],
                    scalar1=1.0 / SW, scalar2=None, op0=OP.mult)

            def qk_unit(j):  # qk = scale_c * (Wk^T q + Wk^T bq), 512 queries
                sl = slice(j * 512, (j + 1) * 512)
                pk = ps_s.tile([128, 2, 512], f32, tag="s", name=f"pqk{j}")
                for ci in range(2):
                    nc.tensor.matmul(
                        pk[:, ci, :],
                        lhsT=wkt8[:, :, ci * 128:(ci + 1) * 128],
                        rhs=qt[:, :, sl], start=True, stop=True,
                        perf_mode=PM.DoubleRow)
                for ci in range(2):
                    nc.vector.tensor_scalar(
                        out=qk[:, ci, sl], in0=pk[:, ci, :],
                        scalar1=qkbias[:, ci, :], scalar2=sc16[:, ci, :],
                        op0=OP.add, op1=OP.mult)

            state = {}

            def s_pair(j, p):
                sl = slice(j * 512, (j + 1) * 512)
                sp = ps_s.tile([128, 2, 512], f32, tag="s", name=f"sp{j}_{p}")
                for par in range(2):
                    mb = 2 * p + par
                    nc.tensor.matmul(
                        sp[:, par, :],
                        lhsT=xf8[:, :, mb * 128:(mb + 1) * 128],
                        rhs=qk[:, :, sl], start=True, stop=True,
                        perf_mode=PM.DoubleRow)
                return sp

            def xe_r(j, p, et):
                po, pr = state["po"], state["pr"]
                for ci in range(2):
                    nc.tensor.matmul(
                        po[:, ci, :],
                        lhsT=xt8[:, 2 * p:2 * p + 2, ci * 128:(ci + 1) * 128],
                        rhs=et, start=(p == 0), stop=(p == NP - 1),
                        perf_mode=PM.DoubleRow, skip_group_check=True)
                nc.tensor.matmul(
                    pr, lhsT=ones8, rhs=et,
                    start=(p == 0), stop=(p == NP - 1),
                    perf_mode=PM.DoubleRow, skip_group_check=True)

            def alloc_por(j):
                state["po"] = ps_o.tile([128, 2, 512], f32, tag="o",
                                        name=f"po{j}")
                state["pr"] = ps_r.tile([128, 512], f32, tag="r",
                                        name=f"pr{j}")

            def attention_all():
                """All 64 key-pair rounds as one flat software pipeline:
                the S pair for round g+1 is always in flight before round
                g's xE/R consumers, across chunk boundaries too (the et pool
                buffers the po/pr handoff)."""
                prev = None
                for g in range(4 * NP):
                    j, p = divmod(g, NP)
                    sp = s_pair(j, p)
                    if prev is not None:
                        jq, pq, etq = prev
                        if pq == 0 and jq >= 1:
                            tail_norm(jq - 1)
                            alloc_por(jq)
                        xe_r(jq, pq, etq)
                    if 2 <= p <= 5 and j >= 1:
                        tail_piece(j - 1, p - 2)
                    if p == 7 and j < 3:
                        qt_unit(j + 1)
                    if p == 11 and j < 3:
                        qk_unit(j + 1)
                    # logit shift keeps exp in e4m3 range (max logit ~8:
                    # e^(8-3)=148 < 240); softmax is shift-invariant
                    et = epool.tile([128, 2, 512], f8, tag="et")
                    nc.scalar.activation(out=et, in_=sp, func=AF.Exp,
                                         scale=1.0 / SW, bias=negc)
                    prev = (j, p, et)
                xe_r(3, NP - 1, prev[2])

            def tail_norm(j):
                """Free po/pr: normalized xE in fp8, 1/R via fast recip."""
                po, pr = state["po"], state["pr"]
                rb = wrk.tile([128, 512], f32, tag="rb")
                nc.vector.reciprocal_approx_fast(out=rb, in_=pr)  # frees pr
                xen = wrk.tile([128, 2, 512], f8, tag="xen")
                onorm = wrk.tile([128, 2, 512], f32, tag="onorm")
                nc.vector.tensor_copy(out=onorm, in_=po)  # frees po
                for ci in range(2):
                    nc.vector.tensor_tensor(
                        out=xen[:, ci, :], in0=onorm[:, ci, :], in1=rb,
                        op=OP.mult)
                state[f"xen{j}"] = xen

            def final_tail(j):
                """Last chunk's tail: nothing overlaps it, so pipeline it in
                query halves through the now-free ps_s banks."""
                po, pr = state["po"], state["pr"]
                sl0 = j * 512
                for h in range(2):
                    q = slice(h * 256, (h + 1) * 256)
                    rb = wrk.tile([128, 256], f32, tag="rbh", name=f"rb{h}")
                    nc.vector.reciprocal_approx_fast(out=rb, in_=pr[:, q])
                    xen = wrk.tile([128, 2, 256], f8, tag="xenh",
                                   name=f"xen{h}")
                    for ci in range(2):
                        nc.vector.tensor_tensor(
                            out=xen[:, ci, :], in0=po[:, ci, q], in1=rb,
                            op=OP.mult)
                    attn8 = wrk.tile([128, 2, 256], f8, tag="attnh",
                                     name=f"attn{h}")
                    for cb in range(2):
                        pa = ps_s.tile([128, 2, 512], f32, tag="s",
                                       name=f"fpa{h}_{cb}")
                        nc.tensor.matmul(
                            pa[:, 0, 0:256],
                            lhsT=w8[:, :, 2 * C + cb * 128:
                                    2 * C + (cb + 1) * 128],
                            rhs=xen, start=True, stop=True,
                            perf_mode=PM.DoubleRow)
                        # ACT is idle post-exp: drain there, off DVE's chain
                        nc.scalar.activation(
                            out=attn8[:, cb, :], in_=pa[:, 0, 0:256],
                            func=AF.Identity, scale=1.0 / SW,
                            bias=bias2[:, cb, 1:2])
                    for co in range(2):
                        pp = ps_s.tile([128, 2, 512], f32, tag="s",
                                       name=f"fpp{h}_{co}")
                        nc.tensor.matmul(
                            pp[:, 0, 0:256],
                            lhsT=w8[:, :, 3 * C + co * 128:
                                    3 * C + (co + 1) * 128],
                            rhs=attn8, start=True, stop=True,
                            perf_mode=PM.DoubleRow)
                        outt = wrk.tile([128, 256], f32, tag="outth",
                                        name=f"outt{h}_{co}")
                        nc.vector.scalar_tensor_tensor(
                            out=outt, in0=pp[:, 0, 0:256], scalar=1.0 / SW,
                            in1=xo[:, co, sl0 + h * 256:sl0 + (h + 1) * 256],
                            op0=OP.mult, op1=OP.add)
                        eng = nc.sync if co == 0 else nc.scalar
                        eng.dma_start(
                            out=d_out.ap()[:, co,
                                           sl0 + h * 256:sl0 + (h + 1) * 256],
                            in_=outt)

            def tail_piece(j, step):
                """One step of: attn = Wv xEn + bv, out = Wp attn/16 + xo.
                Split into 4 single-matmul pieces so the lone ps_x bank's
                drain latency hides under the pair cadence."""
                sl = slice(j * 512, (j + 1) * 512)
                if step == 0:
                    state[f"attn{j}"] = wrk.tile([128, 2, 512], f8,
                                                 tag="attn8",
                                                 name=f"attn{j}")
                if step < 2:
                    cb = step
                    xen = state[f"xen{j}"]
                    pa = ps_x.tile([128, 512], f32, tag="x", name=f"pa{j}_{cb}")
                    nc.tensor.matmul(
                        pa, lhsT=w8[:, :, 2 * C + cb * 128:
                                    2 * C + (cb + 1) * 128],
                        rhs=xen, start=True, stop=True,
                        perf_mode=PM.DoubleRow)
                    nc.vector.tensor_scalar(
                        out=state[f"attn{j}"][:, cb, :], in0=pa,
                        scalar1=1.0 / SW, scalar2=bias2[:, cb, 1:2],
                        op0=OP.mult, op1=OP.add)
                else:
                    co = step - 2
                    pp = ps_x.tile([128, 512], f32, tag="x", name=f"pp{j}_{co}")
                    nc.tensor.matmul(
                        pp, lhsT=w8[:, :, 3 * C + co * 128:
                                    3 * C + (co + 1) * 128],
                        rhs=state[f"attn{j}"], start=True, stop=True,
                        perf_mode=PM.DoubleRow)
                    outt = wrk.tile([128, 512], f32, tag="outt")
                    nc.vector.scalar_tensor_tensor(
                        out=outt, in0=pp, scalar=1.0 / SW,
                        in1=xo[:, co, sl], op0=OP.mult, op1=OP.add)
                    nc.sync.dma_start(out=d_out.ap()[:, co, sl], in_=outt)
                    if co == 1:
                        state.pop(f"xen{j}")
                        state.pop(f"attn{j}")

            def tail_proj(j):
                for step in range(4):
                    tail_piece(j, step)

            # ---- one flat attention pipeline over all 64 pair rounds ----
            qk_unit(0)
            alloc_por(0)
            attention_all()
            final_tail(3)

    nc.compile()
    _cache["nc"] = nc
    return nc


def _prep_maps(x, gn_w, gn_b, qkv_w, qkv_b, proj_w, proj_b):
    """Host-side sharding + layout prep. Returns list of 8 in_maps."""
    import ml_dtypes
    f8 = ml_dtypes.float8_e4m3
    bf16 = ml_dtypes.bfloat16

    x = np.asarray(x, np.float32)
    qkv_w = np.asarray(qkv_w, np.float32)
    qkv_b = np.asarray(qkv_b, np.float32)
    proj_w = np.asarray(proj_w, np.float32)
    proj_b = np.asarray(proj_b, np.float32)
    gn_w = np.asarray(gn_w, np.float32)
    gn_b = np.asarray(gn_b, np.float32)

    def chunked(a):  # [256, ...] -> [128, 2, ...]
        return np.ascontiguousarray(a.reshape(2, 128, *a.shape[1:]).transpose(
            1, 0, *range(2, a.ndim + 1)))

    wq = qkv_w[0:C].T
    wk = qkv_w[C:2 * C].T
    wv = qkv_w[2 * C:3 * C].T
    wp = proj_w.T
    wall = chunked(np.concatenate([wq, wk, wv, wp], axis=1)).astype(bf16)
    # wkt: Wk as [c_out, c_in], raw (no GN scale), pre-scaled by 16 for fp8
    wkt = chunked(qkv_w[C:2 * C] * SW).astype(f8)
    rbias = proj_w @ qkv_b[2 * C:3 * C] + proj_b   # v-bias fold + proj bias
    smalls = chunked(np.stack([qkv_b[0:C], qkv_b[C:2 * C], gn_b], axis=1))

    cidx = np.arange(C)
    ag_full = (cidx[:, None] // CPG == np.arange(G)[None, :]).astype(np.float32)
    ag = chunked(ag_full / CPG)                     # [128, 2, G], carries 1/8
    bg_full = ag_full * gn_w[:, None]               # carries gn_w
    bg = np.ascontiguousarray(
        bg_full.reshape(2, 128, G).transpose(2, 0, 1))  # [G, 2, 128]

    maps = []
    for core in range(8):
        b, half = core // 2, core % 2
        xflat = x[b].reshape(C, HW)
        own = xflat[:, half * NH:(half + 1) * NH]
        other = xflat[:, (1 - half) * NH:(2 - half) * NH]
        xroll = np.concatenate([own, other], axis=1)  # own queries first
        xt = np.ascontiguousarray(
            xroll.T.reshape(MB, 128, C).transpose(1, 0, 2))
        xo = own + rbias[:, None]
        maps.append({
            "xf": chunked(xroll).astype(f8),
            "xt": xt.astype(f8),
            "xo": chunked(xo),
            "w": wall, "wkt": wkt, "sb": smalls, "ag": ag, "bg": bg,
        })
    return maps


def kernel(x, gn_w, gn_b, qkv_w, qkv_b, proj_w, proj_b):
    import concourse.bass_utils as bu
    nc = build_nc()
    maps = _prep_maps(x, gn_w, gn_b, qkv_w, qkv_b, proj_w, proj_b)
    res = bu.run_bass_kernel_spmd(nc, maps, core_ids=list(range(8)))
    out = np.empty((B, C, HW), np.float32)
    for core in range(8):
        b, half = core // 2, core % 2
        o = res.results[core]["out"]                # [128, 2, NH]
        out[b, :, half * NH:(half + 1) * NH] = \
            o.transpose(1, 0, 2).reshape(C, NH)
    return out.reshape(B, C, 64, 64)


# revision 41
# speedup vs baseline: 1.0475x; 1.0475x over previous
"""AttentionBlock (B=4, C=256, H=W=64) on 8 Trainium2 NeuronCores.

Sharding: data-parallel over (batch, query-half): core i handles batch i//2,
query pixels [half*2048, (half+1)*2048), half = i%2. GroupNorm stats are
computed per batch element (duplicated across the pair, cheap); the O(N^2)
attention work is fully sharded 8 ways. No collectives.

v5: all large matmuls are fp8e4 DoubleRow (K=256 per 512-row instruction).
k and v are never materialized -- by associativity the attention runs on
raw fp8 x from both ends:
    S  = k^T q       = x^T (Wk^T q)           (qk made once per chunk)
    O  = v E         = Wv (x E) + bv R        (xE accumulated in PSUM)
so the inner loop is just S(pair) -> exp -> xE/R accumulate, identical for
all 64 pairs, with zero per-pair weight/bias traffic. The GN channel scale
rides the per-partition qk/q drains, bv/bq fold into per-partition drain
biases, and the k-side bias is dropped outright (a per-query logit shift is
softmax-invariant). exp runs on ACT in [128,1024] chunks into fp8 with a -3
logit shift (e4m3 range), software-pipelined one S-pair ahead of the xE/R
consumers. 1/R uses reciprocal_approx_fast on the PE-broadcast R block.
Host ships x twice in fp8 ([cin, pix] and [pix, cin], query half first),
the residual base x+rbias in f32, and weights in bf16 (GN fold + 16x fp8
cast on device; Wk^T additionally raw fp8).
"""

import numpy as np

B, C, HW = 4, 256, 4096
NH = 2048            # query pixels per core
G, CPG = 32, 8       # groups, channels per group
EPS = 1e-5
MB = HW // 128       # 32 key blocks
NP = MB // 2         # 16 key-block pairs
SW = 16.0            # fp8 weight pre-scale

_cache = {}


def build_nc():
    """Build (and cache) the Bass module."""
    if "nc" in _cache:
        return _cache["nc"]
    import concourse.tile as tile
    from concourse import bacc, mybir

    f32 = mybir.dt.float32
    bf16 = mybir.dt.bfloat16
    f8 = mybir.dt.float8e4
    AF = mybir.ActivationFunctionType
    OP = mybir.AluOpType
    PM = mybir.MatmulPerfMode

    nc = bacc.Bacc("TRN2", target_bir_lowering=False, debug=False,
                   enable_asserts=False, num_devices=8)

    # ---- DRAM I/O (host preps everything into device layout) ----
    d_xf = nc.dram_tensor("xf", [128, 2, HW], f8, kind="ExternalInput")
    d_xt = nc.dram_tensor("xt", [128, MB, C], f8, kind="ExternalInput")
    d_xo = nc.dram_tensor("xo", [128, 2, NH], f32, kind="ExternalInput")
    d_w = nc.dram_tensor("w", [128, 2, 4 * C], bf16, kind="ExternalInput")
    d_wkt = nc.dram_tensor("wkt", [128, 2, C], f8, kind="ExternalInput")
    d_sb = nc.dram_tensor("sb", [128, 2, 3], f32, kind="ExternalInput")
    d_ag = nc.dram_tensor("ag", [128, 2, G], f32, kind="ExternalInput")
    d_bg = nc.dram_tensor("bg", [G, 2, 128], f32, kind="ExternalInput")
    d_out = nc.dram_tensor("out", [128, 2, NH], f32, kind="ExternalOutput")

    with tile.TileContext(nc) as tc:
        with (
            tc.tile_pool(name="big", bufs=1) as big,
            tc.tile_pool(name="cst", bufs=1) as cst,
            tc.tile_pool(name="wrk", bufs=2) as wrk,
            tc.tile_pool(name="epool", bufs=4) as epool,
            tc.tile_pool(name="gnp", bufs=1) as gnp,
            tc.tile_pool(name="ps_s", bufs=2, space="PSUM") as ps_s,
            tc.tile_pool(name="ps_o", bufs=1, space="PSUM") as ps_o,
            tc.tile_pool(name="ps_r", bufs=1, space="PSUM") as ps_r,
            tc.tile_pool(name="ps_x", bufs=1, space="PSUM") as ps_x,
        ):
            # ---- input DMAs: x first (gates GN stats); big descriptors,
            # few triggers; params on scalar after x ----
            # sync ring: x h0 quarters, xt h0, GN params, xo (needed late).
            # scalar ring: ci0-h1 x, weights, ci1-h1 x, xt h1 -- ordered so
            # each piece lands just before its consumer needs it.
            xf8 = big.tile([128, 2, HW], f8, tag="xf8")
            bstat = gnp.tile([128, 2, 8, 6], f32, tag="bstat")
            xt8 = big.tile([128, MB, C], f8, tag="xt8")
            wall = cst.tile([128, 2, 4 * C], bf16, tag="wall")
            wkt8 = cst.tile([128, 2, C], f8, tag="wkt8")

            def xf_dma(ci, h, eng):
                for q in range(2):
                    sl = slice(h * NH + q * 1024, h * NH + (q + 1) * 1024)
                    eng.dma_start(out=xf8[:, ci, sl], in_=d_xf.ap()[:, ci, sl])

            xf_dma(0, 0, nc.sync)
            xf_dma(1, 0, nc.sync)
            xf_dma(0, 1, nc.scalar)
            nc.scalar.dma_start(out=wall, in_=d_w.ap())
            nc.scalar.dma_start(out=wkt8, in_=d_wkt.ap())
            xf_dma(1, 1, nc.scalar)
            for h in range(2):
                sl = slice(h * MB // 2, (h + 1) * MB // 2)
                eng = nc.sync if h == 0 else nc.scalar
                eng.dma_start(out=xt8[:, sl, :], in_=d_xt.ap()[:, sl, :])
            # stats in arrival order: ci0 fully, then ci1 (its scalar-ring
            # h1 quarters land last)
            for ci in range(2):
                for j in range(8):
                    nc.vector.bn_stats(
                        out=bstat[:, ci, j, :],
                        in_=xf8[:, ci, j * 512:(j + 1) * 512])
            smalls = cst.tile([128, 2, 3], f32, tag="smalls")
            nc.sync.dma_start(out=smalls, in_=d_sb.ap())
            qb = smalls[:, :, 0:1]
            gb = smalls[:, :, 2:3]
            ag = cst.tile([128, 2, G], f32, tag="ag")
            nc.sync.dma_start(out=ag, in_=d_ag.ap())
            bg = cst.tile([G, 2, 128], f32, tag="bg")
            nc.sync.dma_start(out=bg, in_=d_bg.ap())
            xo = big.tile([128, 2, NH], f32, tag="xo")
            for ci in range(2):
                nc.sync.dma_start(out=xo[:, ci, :], in_=d_xo.ap()[:, ci, :])

            epst = cst.tile([G, 1], f32, tag="epst")
            nc.vector.memset(epst, EPS)
            negc = cst.tile([128, 1], f32, tag="negc")  # softmax logit shift
            nc.vector.memset(negc, -3.0)
            # R lhsT (DoubleRow, M=128: R lands pre-broadcast on all rows)
            ones8 = cst.tile([128, 2, 128], f8, tag="ones8")
            nc.vector.memset(ones8, 1.0)

            # ---- GroupNorm stats aggregation ----
            stats2 = gnp.tile([128, 2, 2], f32, tag="stats2")  # (mean, E[x^2])
            tmp1 = gnp.tile([128, 1], f32, tag="tmp1")
            for ci in range(2):
                nc.vector.bn_aggr(out=stats2[:, ci, :], in_=bstat[:, ci, :, :])
                nc.vector.tensor_tensor(
                    out=tmp1, in0=stats2[:, ci, 0:1], in1=stats2[:, ci, 0:1],
                    op=OP.mult)
                nc.vector.tensor_tensor(
                    out=stats2[:, ci, 1:2], in0=stats2[:, ci, 1:2], in1=tmp1,
                    op=OP.add)
            # group sums across partitions: [G, 2] = sum_ci ag[ci]^T stats2[ci]
            pg = ps_x.tile([G, 2], f32, tag="x")
            for ci in range(2):
                nc.tensor.matmul(pg, lhsT=ag[:, ci, :], rhs=stats2[:, ci, :],
                                 start=(ci == 0), stop=(ci == 1))
            # ag carries 1/CPG so pg is directly (mean_g, E[x^2]_g)
            pgs = gnp.tile([G, 2], f32, tag="pgs")
            nc.vector.tensor_copy(out=pgs, in_=pg)
            gst = gnp.tile([G, 4], f32, tag="gst")  # mean^2, var, sd, -
            nc.vector.tensor_tensor(out=gst[:, 0:1], in0=pgs[:, 0:1],
                                    in1=pgs[:, 0:1], op=OP.mult)
            nc.vector.tensor_tensor(out=gst[:, 1:2], in0=pgs[:, 1:2],
                                    in1=gst[:, 0:1], op=OP.subtract)
            gfin = gnp.tile([G, 2], f32, tag="gfin")  # (rstd_g, mean_g*rstd_g)
            nc.scalar.activation(out=gst[:, 2:3], in_=gst[:, 1:2],
                                 func=AF.Sqrt, bias=epst)
            nc.vector.reciprocal(out=gfin[:, 0:1], in_=gst[:, 2:3])
            nc.vector.tensor_tensor(out=gfin[:, 1:2], in0=pgs[:, 0:1],
                                    in1=gfin[:, 0:1], op=OP.mult)
            # bg carries gn_w, so pbc = (scale_c, mean_c*scale_c);
            # bias_c = gn_b - mean_c*scale_c
            scbc = gnp.tile([128, 2, 2], f32, tag="scbc")
            for ci in range(2):
                pbc = ps_x.tile([128, 2], f32, tag="x")
                nc.tensor.matmul(pbc, lhsT=bg[:, ci, :], rhs=gfin,
                                 start=True, stop=True)
                nc.vector.tensor_copy(out=scbc[:, ci, 0:1], in_=pbc[:, 0:1])
                nc.vector.tensor_tensor(out=scbc[:, ci, 1:2], in0=gb[:, ci, :],
                                        in1=pbc[:, 1:2], op=OP.subtract)

            # ---- fp8 weights (gate q/qk production):
            # W8 = W * scale_c * 16 (q,v), W * 16 (proj)
            w8 = cst.tile([128, 2, 4 * C], f8, tag="w8")
            for ci in range(2):
                nc.vector.tensor_scalar(
                    out=w8[:, ci, 0:3 * C], in0=wall[:, ci, 0:3 * C],
                    scalar1=scbc[:, ci, 0:1], scalar2=SW,
                    op0=OP.mult, op1=OP.mult)
                nc.vector.tensor_scalar(
                    out=w8[:, ci, 3 * C:4 * C], in0=wall[:, ci, 3 * C:4 * C],
                    scalar1=SW, scalar2=None, op0=OP.mult)

            qt = big.tile([128, 2, NH], f8, tag="qt")
            qk = big.tile([128, 2, NH], f8, tag="qk")

            def qt_unit(j):  # q for 512 queries: Wq' x (bias rides in qk)
                sl = slice(j * 512, (j + 1) * 512)
                pq = ps_s.tile([128, 2, 512], f32, tag="s", name=f"pq{j}")
                for cb in range(2):
                    nc.tensor.matmul(
                        pq[:, cb, :],
                        lhsT=w8[:, :, cb * 128:(cb + 1) * 128],
                        rhs=xf8[:, :, sl], start=True, stop=True,
                        perf_mode=PM.DoubleRow)
                nc.vector.tensor_scalar(
                    out=qt[:, :, sl], in0=pq, scalar1=1.0 / SW, scalar2=None,
                    op0=OP.mult)

            qt_unit(0)

            # ---- per-partition drain biases from the GN fold (bias_c on
            # unscaled bf16 W). The k-side bias shifts every logit of a query
            # equally -- softmax-invariant -- so it is dropped.
            bcr = gnp.tile([128, 2, 2], bf16, tag="bcr")  # bias_c, 2 copies
            for ci in range(2):
                nc.vector.tensor_copy(out=bcr[:, ci, 0:1],
                                      in_=scbc[:, ci, 1:2])
                nc.vector.tensor_copy(out=bcr[:, ci, 1:2],
                                      in_=scbc[:, ci, 1:2])
            # bias2[:, cb, 0] = qb + Wq @ bias_c ; bias2[:, cb, 1] = Wv @ bias_c
            bias2 = gnp.tile([128, 2, 2], f32, tag="bias2")
            for wi, woff in ((0, 0), (1, 2 * C)):
                for cb in range(2):
                    pbias = ps_x.tile([128, 2], f32, tag="x")
                    for ci in range(2):
                        nc.tensor.matmul(
                            pbias,
                            lhsT=wall[:, ci, woff + cb * 128:
                                      woff + (cb + 1) * 128],
                            rhs=bcr[:, ci, :], start=(ci == 0), stop=(ci == 1))
                    if wi == 0:
                        nc.vector.tensor_tensor(
                            out=bias2[:, cb, 0:1], in0=pbias[:, 0:1],
                            in1=qb[:, cb, :], op=OP.add)
                    else:
                        nc.vector.tensor_copy(out=bias2[:, cb, 1:2],
                                              in_=pbias[:, 0:1])
            # fold the q bias into the qk drain instead of the q drain:
            # qk = s/16 * Wk16^T (q0 + bq) = (pk + qkb) * sc16,
            # qkb = Wk16^T bq16 / 16. qt then drains biasless in one op.
            bqr8 = gnp.tile([128, 2, 2], f8, tag="bqr8")
            for cb in range(2):
                for k in range(2):
                    nc.vector.tensor_scalar(
                        out=bqr8[:, cb, k:k + 1], in0=bias2[:, cb, 0:1],
                        scalar1=SW, scalar2=None, op0=OP.mult)
            qkbias = gnp.tile([128, 2, 1], f32, tag="qkbias")
            sc16 = gnp.tile([128, 2, 1], f32, tag="sc16")
            for ci in range(2):
                pqkb = ps_x.tile([128, 2], f32, tag="x")
                nc.tensor.matmul(
                    pqkb, lhsT=wkt8[:, :, ci * 128:(ci + 1) * 128],
                    rhs=bqr8, start=True, stop=True, perf_mode=PM.DoubleRow)
                nc.vector.tensor_scalar(
                    out=qkbias[:, ci, :], in0=pqkb[:, 0:1],
                    scalar1=1.0 / SW, scalar2=None, op0=OP.mult)
                nc.vector.tensor_scalar(
                    out=sc16[:, ci, :], in0=scbc[:, ci, 0:1],
                    scalar1=1.0 / SW, scalar2=None, op0=OP.mult)

            def qk_unit(j):  # qk = scale_c * (Wk^T q + Wk^T bq), 512 queries
                sl = slice(j * 512, (j + 1) * 512)
                pk = ps_s.tile([128, 2, 512], f32, tag="s", name=f"pqk{j}")
                for ci in range(2):
                    nc.tensor.matmul(
                        pk[:, ci, :],
                        lhsT=wkt8[:, :, ci * 128:(ci + 1) * 128],
                        rhs=qt[:, :, sl], start=True, stop=True,
                        perf_mode=PM.DoubleRow)
                for ci in range(2):
                    nc.vector.tensor_scalar(
                        out=qk[:, ci, sl], in0=pk[:, ci, :],
                        scalar1=qkbias[:, ci, :], scalar2=sc16[:, ci, :],
                        op0=OP.add, op1=OP.mult)

            state = {}

            def s_pair(j, p):
                sl = slice(j * 512, (j + 1) * 512)
                sp = ps_s.tile([128, 2, 512], f32, tag="s", name=f"sp{j}_{p}")
                for par in range(2):
                    mb = 2 * p + par
                    nc.tensor.matmul(
                        sp[:, par, :],
                        lhsT=xf8[:, :, mb * 128:(mb + 1) * 128],
                        rhs=qk[:, :, sl], start=True, stop=True,
                        perf_mode=PM.DoubleRow)
                return sp

            def xe_r(j, p, et):
                po, pr = state["po"], state["pr"]
                for ci in range(2):
                    nc.tensor.matmul(
                        po[:, ci, :],
                        lhsT=xt8[:, 2 * p:2 * p + 2, ci * 128:(ci + 1) * 128],
                        rhs=et, start=(p == 0), stop=(p == NP - 1),
                        perf_mode=PM.DoubleRow, skip_group_check=True)
                nc.tensor.matmul(
                    pr, lhsT=ones8, rhs=et,
                    start=(p == 0), stop=(p == NP - 1),
                    perf_mode=PM.DoubleRow, skip_group_check=True)

            def alloc_por(j):
                state["po"] = ps_o.tile([128, 2, 512], f32, tag="o",
                                        name=f"po{j}")
                state["pr"] = ps_r.tile([128, 512], f32, tag="r",
                                        name=f"pr{j}")

            def attention_all():
                """All 64 key-pair rounds as one flat software pipeline:
                the S pair for round g+1 is always in flight before round
                g's xE/R consumers, across chunk boundaries too (the et pool
                buffers the po/pr handoff)."""
                prev = None
                for g in range(4 * NP):
                    j, p = divmod(g, NP)
                    sp = s_pair(j, p)
                    if prev is not None:
                        jq, pq, etq = prev
                        if pq == 0 and jq >= 1:
                            tail_norm(jq - 1)
                            alloc_por(jq)
                        xe_r(jq, pq, etq)
                    if 2 <= p <= 5 and j >= 1:
                        tail_piece(j - 1, p - 2)
                    if p == 7 and j < 3:
                        qt_unit(j + 1)
                    if p == 11 and j < 3:
                        qk_unit(j + 1)
                    # logit shift keeps exp in e4m3 range (max logit ~8:
                    # e^(8-3)=148 < 240); softmax is shift-invariant
                    et = epool.tile([128, 2, 512], f8, tag="et")
                    nc.scalar.activation(out=et, in_=sp, func=AF.Exp,
                                         scale=1.0 / SW, bias=negc)
                    prev = (j, p, et)
                xe_r(3, NP - 1, prev[2])

            def tail_norm(j):
                """Free po/pr: normalized xE in fp8, 1/R via fast recip."""
                po, pr = state["po"], state["pr"]
                rb = wrk.tile([128, 512], f32, tag="rb")
                nc.vector.reciprocal_approx_fast(out=rb, in_=pr)  # frees pr
                xen = wrk.tile([128, 2, 512], f8, tag="xen")
                onorm = wrk.tile([128, 2, 512], f32, tag="onorm")
                nc.vector.tensor_copy(out=onorm, in_=po)  # frees po
                for ci in range(2):
                    nc.vector.tensor_tensor(
                        out=xen[:, ci, :], in0=onorm[:, ci, :], in1=rb,
                        op=OP.mult)
                state[f"xen{j}"] = xen

            def final_tail(j):
                """Last chunk's tail: nothing overlaps it, so pipeline it in
                query halves through the now-free ps_s banks."""
                po, pr = state["po"], state["pr"]
                sl0 = j * 512
                for h in range(2):
                    q = slice(h * 256, (h + 1) * 256)
                    rb = wrk.tile([128, 256], f32, tag="rbh", name=f"rb{h}")
                    nc.vector.reciprocal_approx_fast(out=rb, in_=pr[:, q])
                    xen = wrk.tile([128, 2, 256], f8, tag="xenh",
                                   name=f"xen{h}")
                    for ci in range(2):
                        nc.vector.tensor_tensor(
                            out=xen[:, ci, :], in0=po[:, ci, q], in1=rb,
                            op=OP.mult)
                    attn8 = wrk.tile([128, 2, 256], f8, tag="attnh",
                                     name=f"attn{h}")
                    for cb in range(2):
                        pa = ps_s.tile([128, 2, 512], f32, tag="s",
                                       name=f"fpa{h}_{cb}")
                        nc.tensor.matmul(
                            pa[:, 0, 0:256],
                            lhsT=w8[:, :, 2 * C + cb * 128:
                                    2 * C + (cb + 1) * 128],
                            rhs=xen, start=True, stop=True,
                            perf_mode=PM.DoubleRow)
                        # ACT is idle post-exp: drain there, off DVE's chain
                        nc.scalar.activation(
                            out=attn8[:, cb, :], in_=pa[:, 0, 0:256],
                            func=AF.Identity, scale=1.0 / SW,
                            bias=bias2[:, cb, 1:2])
                    for co in range(2):
                        pp = ps_s.tile([128, 2, 512], f32, tag="s",
                                       name=f"fpp{h}_{co}")
                        nc.tensor.matmul(
                            pp[:, 0, 0:256],
                            lhsT=w8[:, :, 3 * C + co * 128:
                                    3 * C + (co + 1) * 128],
                            rhs=attn8, start=True, stop=True,
                            perf_mode=PM.DoubleRow)
                        outt = wrk.tile([128, 256], f32, tag="outth",
                                        name=f"outt{h}_{co}")
                        nc.vector.scalar_tensor_tensor(
                            out=outt, in0=pp[:, 0, 0:256], scalar=1.0 / SW,
                            in1=xo[:, co, sl0 + h * 256:sl0 + (h + 1) * 256],
                            op0=OP.mult, op1=OP.add)
                        eng = nc.sync if co == 0 else nc.scalar
                        eng.dma_start(
                            out=d_out.ap()[:, co,
                                           sl0 + h * 256:sl0 + (h + 1) * 256],
                            in_=outt)

            def tail_piece(j, step):
                """One step of: attn = Wv xEn + bv, out = Wp attn/16 + xo.
                Split into 4 single-matmul pieces so the lone ps_x bank's
                drain latency hides under the pair cadence."""
                sl = slice(j * 512, (j + 1) * 512)
                if step == 0:
                    state[f"attn{j}"] = wrk.tile([128, 2, 512], f8,
                                                 tag="attn8",
                                                 name=f"attn{j}")
                if step < 2:
                    cb = step
                    xen = state[f"xen{j}"]
                    pa = ps_x.tile([128, 512], f32, tag="x", name=f"pa{j}_{cb}")
                    nc.tensor.matmul(
                        pa, lhsT=w8[:, :, 2 * C + cb * 128:
                                    2 * C + (cb + 1) * 128],
                        rhs=xen, start=True, stop=True,
                        perf_mode=PM.DoubleRow)
                    nc.vector.tensor_scalar(
                        out=state[f"attn{j}"][:, cb, :], in0=pa,
                        scalar1=1.0 / SW, scalar2=bias2[:, cb, 1:2],
                        op0=OP.mult, op1=OP.add)
                else:
                    co = step - 2
                    pp = ps_x.tile([128, 512], f32, tag="x", name=f"pp{j}_{co}")
                    nc.tensor.matmul(
                        pp, lhsT=w8[:, :, 3 * C + co * 128:
                                    3 * C + (co + 1) * 128],
                        rhs=state[f"attn{j}"], start=True, stop=True,
                        perf_mode=PM.DoubleRow)
                    outt = wrk.tile([128, 512], f32, tag="outt")
                    nc.vector.scalar_tensor_tensor(
                        out=outt, in0=pp, scalar=1.0 / SW,
                        in1=xo[:, co, sl], op0=OP.mult, op1=OP.add)
                    nc.sync.dma_start(out=d_out.ap()[:, co, sl], in_=outt)
                    if co == 1:
                        state.pop(f"xen{j}")
                        state.pop(f"attn{j}")

            def tail_proj(j):
                for step in range(4):
                    tail_piece(j, step)

            # ---- one flat attention pipeline over all 64 pair rounds ----
            qk_unit(0)
            alloc_por(0)
            attention_all()
            final_tail(3)

    nc.compile()
    _cache["nc"] = nc
    return nc


def _prep_maps(x, gn_w, gn_b, qkv_w, qkv_b, proj_w, proj_b):
    """Host-side sharding + layout prep. Returns list of 8 in_maps."""
    import ml_dtypes
    f8 = ml_dtypes.float8_e4m3
    bf16 = ml_dtypes.bfloat16

    x = np.asarray(x, np.float32)
    qkv_w = np.asarray(qkv_w, np.float32)
    qkv_b = np.asarray(qkv_b, np.float32)
    proj_w = np.asarray(proj_w, np.float32)
    proj_b = np.asarray(proj_b, np.float32)
    gn_w = np.asarray(gn_w, np.float32)
    gn_b = np.asarray(gn_b, np.float32)

    def chunked(a):  # [256, ...] -> [128, 2, ...]
        return np.ascontiguousarray(a.reshape(2, 128, *a.shape[1:]).transpose(
            1, 0, *range(2, a.ndim + 1)))

    wq = qkv_w[0:C].T
    wk = qkv_w[C:2 * C].T
    wv = qkv_w[2 * C:3 * C].T
    wp = proj_w.T
    wall = chunked(np.concatenate([wq, wk, wv, wp], axis=1)).astype(bf16)
    # wkt: Wk as [c_out, c_in], raw (no GN scale), pre-scaled by 16 for fp8
    wkt = chunked(qkv_w[C:2 * C] * SW).astype(f8)
    rbias = proj_w @ qkv_b[2 * C:3 * C] + proj_b   # v-bias fold + proj bias
    smalls = chunked(np.stack([qkv_b[0:C], qkv_b[C:2 * C], gn_b], axis=1))

    cidx = np.arange(C)
    ag_full = (cidx[:, None] // CPG == np.arange(G)[None, :]).astype(np.float32)
    ag = chunked(ag_full / CPG)                     # [128, 2, G], carries 1/8
    bg_full = ag_full * gn_w[:, None]               # carries gn_w
    bg = np.ascontiguousarray(
        bg_full.reshape(2, 128, G).transpose(2, 0, 1))  # [G, 2, 128]

    maps = []
    for core in range(8):
        b, half = core // 2, core % 2
        xflat = x[b].reshape(C, HW)
        own = xflat[:, half * NH:(half + 1) * NH]
        other = xflat[:, (1 - half) * NH:(2 - half) * NH]
        xroll = np.concatenate([own, other], axis=1)  # own queries first
        xt = np.ascontiguousarray(
            xroll.T.reshape(MB, 128, C).transpose(1, 0, 2))
        xo = own + rbias[:, None]
        maps.append({
            "xf": chunked(xroll).astype(f8),
            "xt": xt.astype(f8),
            "xo": chunked(xo),
            "w": wall, "wkt": wkt, "sb": smalls, "ag": ag, "bg": bg,
        })
    return maps


def kernel(x, gn_w, gn_b, qkv_w, qkv_b, proj_w, proj_b):
    import concourse.bass_utils as bu
    nc = build_nc()
    maps = _prep_maps(x, gn_w, gn_b, qkv_w, qkv_b, proj_w, proj_b)
    res = bu.run_bass_kernel_spmd(nc, maps, core_ids=list(range(8)))
    out = np.empty((B, C, HW), np.float32)
    for core in range(8):
        b, half = core // 2, core % 2
        o = res.results[core]["out"]                # [128, 2, NH]
        out[b, :, half * NH:(half + 1) * NH] = \
            o.transpose(1, 0, 2).reshape(C, NH)
    return out.reshape(B, C, 64, 64)


# revision 42
# speedup vs baseline: 1.0559x; 1.0080x over previous
"""AttentionBlock (B=4, C=256, H=W=64) on 8 Trainium2 NeuronCores.

Sharding: data-parallel over (batch, query-half): core i handles batch i//2,
query pixels [half*2048, (half+1)*2048), half = i%2. GroupNorm stats are
computed per batch element (duplicated across the pair, cheap); the O(N^2)
attention work is fully sharded 8 ways. No collectives.

v5: all large matmuls are fp8e4 DoubleRow (K=256 per 512-row instruction).
k and v are never materialized -- by associativity the attention runs on
raw fp8 x from both ends:
    S  = k^T q       = x^T (Wk^T q)           (qk made once per chunk)
    O  = v E         = Wv (x E) + bv R        (xE accumulated in PSUM)
so the inner loop is just S(pair) -> exp -> xE/R accumulate, identical for
all 64 pairs, with zero per-pair weight/bias traffic. The GN channel scale
rides the per-partition qk/q drains, bv/bq fold into per-partition drain
biases, and the k-side bias is dropped outright (a per-query logit shift is
softmax-invariant). exp runs on ACT in [128,1024] chunks into fp8 with a -3
logit shift (e4m3 range), software-pipelined one S-pair ahead of the xE/R
consumers. 1/R uses reciprocal_approx_fast on the PE-broadcast R block.
Host ships x twice in fp8 ([cin, pix] and [pix, cin], query half first),
the residual base x+rbias in f32, and weights in bf16 (GN fold + 16x fp8
cast on device; Wk^T additionally raw fp8).
"""

import numpy as np

B, C, HW = 4, 256, 4096
NH = 2048            # query pixels per core
G, CPG = 32, 8       # groups, channels per group
EPS = 1e-5
MB = HW // 128       # 32 key blocks
NP = MB // 2         # 16 key-block pairs
SW = 16.0            # fp8 weight pre-scale

_cache = {}


def build_nc():
    """Build (and cache) the Bass module."""
    if "nc" in _cache:
        return _cache["nc"]
    import concourse.tile as tile
    from concourse import bacc, mybir

    f32 = mybir.dt.float32
    bf16 = mybir.dt.bfloat16
    f8 = mybir.dt.float8e4
    AF = mybir.ActivationFunctionType
    OP = mybir.AluOpType
    PM = mybir.MatmulPerfMode

    nc = bacc.Bacc("TRN2", target_bir_lowering=False, debug=False,
                   enable_asserts=False, num_devices=8)

    # ---- DRAM I/O (host preps everything into device layout) ----
    d_xf = nc.dram_tensor("xf", [128, 2, HW], f8, kind="ExternalInput")
    d_xt = nc.dram_tensor("xt", [128, MB, C], f8, kind="ExternalInput")
    d_xo = nc.dram_tensor("xo", [128, 2, NH], f32, kind="ExternalInput")
    d_w = nc.dram_tensor("w", [128, 2, 2 * C], bf16, kind="ExternalInput")
    d_wkt = nc.dram_tensor("wkt", [128, 2, C], f8, kind="ExternalInput")
    d_sb = nc.dram_tensor("sb", [128, 2, 3], f32, kind="ExternalInput")
    d_ag = nc.dram_tensor("ag", [128, 2, G], f32, kind="ExternalInput")
    d_bg = nc.dram_tensor("bg", [G, 2, 128], f32, kind="ExternalInput")
    d_out = nc.dram_tensor("out", [128, 2, NH], f32, kind="ExternalOutput")

    with tile.TileContext(nc) as tc:
        with (
            tc.tile_pool(name="big", bufs=1) as big,
            tc.tile_pool(name="cst", bufs=1) as cst,
            tc.tile_pool(name="wrk", bufs=2) as wrk,
            tc.tile_pool(name="epool", bufs=4) as epool,
            tc.tile_pool(name="gnp", bufs=1) as gnp,
            tc.tile_pool(name="ps_s", bufs=2, space="PSUM") as ps_s,
            tc.tile_pool(name="ps_o", bufs=1, space="PSUM") as ps_o,
            tc.tile_pool(name="ps_r", bufs=1, space="PSUM") as ps_r,
            tc.tile_pool(name="ps_x", bufs=1, space="PSUM") as ps_x,
        ):
            # ---- input DMAs: x first (gates GN stats); big descriptors,
            # few triggers; params on scalar after x ----
            # sync ring: x h0 quarters, xt h0, GN params, xo (needed late).
            # scalar ring: ci0-h1 x, weights, ci1-h1 x, xt h1 -- ordered so
            # each piece lands just before its consumer needs it.
            xf8 = big.tile([128, 2, HW], f8, tag="xf8")
            bstat = gnp.tile([128, 2, 8, 6], f32, tag="bstat")
            xt8 = big.tile([128, MB, C], f8, tag="xt8")
            wall = cst.tile([128, 2, 2 * C], bf16, tag="wall")
            wkt8 = cst.tile([128, 2, C], f8, tag="wkt8")

            def xf_dma(ci, h, eng):
                for q in range(2):
                    sl = slice(h * NH + q * 1024, h * NH + (q + 1) * 1024)
                    eng.dma_start(out=xf8[:, ci, sl], in_=d_xf.ap()[:, ci, sl])

            xf_dma(0, 0, nc.sync)
            xf_dma(1, 0, nc.sync)
            xf_dma(0, 1, nc.scalar)
            nc.scalar.dma_start(out=wall, in_=d_w.ap())
            nc.scalar.dma_start(out=wkt8, in_=d_wkt.ap())
            xf_dma(1, 1, nc.scalar)
            for h in range(2):
                sl = slice(h * MB // 2, (h + 1) * MB // 2)
                eng = nc.sync if h == 0 else nc.scalar
                eng.dma_start(out=xt8[:, sl, :], in_=d_xt.ap()[:, sl, :])
            # stats in arrival order: ci0 fully, then ci1 (its scalar-ring
            # h1 quarters land last)
            for ci in range(2):
                for j in range(8):
                    nc.vector.bn_stats(
                        out=bstat[:, ci, j, :],
                        in_=xf8[:, ci, j * 512:(j + 1) * 512])
            smalls = cst.tile([128, 2, 3], f32, tag="smalls")
            nc.sync.dma_start(out=smalls, in_=d_sb.ap())
            qb = smalls[:, :, 0:1]
            gb = smalls[:, :, 2:3]
            ag = cst.tile([128, 2, G], f32, tag="ag")
            nc.sync.dma_start(out=ag, in_=d_ag.ap())
            bg = cst.tile([G, 2, 128], f32, tag="bg")
            nc.sync.dma_start(out=bg, in_=d_bg.ap())
            xo = big.tile([128, 2, NH], f32, tag="xo")
            for ci in range(2):
                nc.sync.dma_start(out=xo[:, ci, :], in_=d_xo.ap()[:, ci, :])

            epst = cst.tile([G, 1], f32, tag="epst")
            nc.vector.memset(epst, EPS)
            negc = cst.tile([128, 1], f32, tag="negc")  # softmax logit shift
            nc.vector.memset(negc, -3.0)
            # R lhsT (DoubleRow, M=128: R lands pre-broadcast on all rows)
            ones8 = cst.tile([128, 2, 128], f8, tag="ones8")
            nc.vector.memset(ones8, 1.0)

            # ---- GroupNorm stats aggregation ----
            stats2 = gnp.tile([128, 2, 2], f32, tag="stats2")  # (mean, E[x^2])
            tmp1 = gnp.tile([128, 1], f32, tag="tmp1")
            for ci in range(2):
                nc.vector.bn_aggr(out=stats2[:, ci, :], in_=bstat[:, ci, :, :])
                nc.vector.tensor_tensor(
                    out=tmp1, in0=stats2[:, ci, 0:1], in1=stats2[:, ci, 0:1],
                    op=OP.mult)
                nc.vector.tensor_tensor(
                    out=stats2[:, ci, 1:2], in0=stats2[:, ci, 1:2], in1=tmp1,
                    op=OP.add)
            # group sums across partitions: [G, 2] = sum_ci ag[ci]^T stats2[ci]
            pg = ps_x.tile([G, 2], f32, tag="x")
            for ci in range(2):
                nc.tensor.matmul(pg, lhsT=ag[:, ci, :], rhs=stats2[:, ci, :],
                                 start=(ci == 0), stop=(ci == 1))
            # ag carries 1/CPG so pg is directly (mean_g, E[x^2]_g)
            pgs = gnp.tile([G, 2], f32, tag="pgs")
            nc.vector.tensor_copy(out=pgs, in_=pg)
            gst = gnp.tile([G, 4], f32, tag="gst")  # mean^2, var, sd, -
            nc.vector.tensor_tensor(out=gst[:, 0:1], in0=pgs[:, 0:1],
                                    in1=pgs[:, 0:1], op=OP.mult)
            nc.vector.tensor_tensor(out=gst[:, 1:2], in0=pgs[:, 1:2],
                                    in1=gst[:, 0:1], op=OP.subtract)
            gfin = gnp.tile([G, 2], f32, tag="gfin")  # (rstd_g, mean_g*rstd_g)
            nc.scalar.activation(out=gst[:, 2:3], in_=gst[:, 1:2],
                                 func=AF.Sqrt, bias=epst)
            nc.vector.reciprocal(out=gfin[:, 0:1], in_=gst[:, 2:3])
            nc.vector.tensor_tensor(out=gfin[:, 1:2], in0=pgs[:, 0:1],
                                    in1=gfin[:, 0:1], op=OP.mult)
            # bg carries gn_w, so pbc = (scale_c, mean_c*scale_c);
            # bias_c = gn_b - mean_c*scale_c
            scbc = gnp.tile([128, 2, 2], f32, tag="scbc")
            for ci in range(2):
                pbc = ps_x.tile([128, 2], f32, tag="x")
                nc.tensor.matmul(pbc, lhsT=bg[:, ci, :], rhs=gfin,
                                 start=True, stop=True)
                nc.vector.tensor_copy(out=scbc[:, ci, 0:1], in_=pbc[:, 0:1])
                nc.vector.tensor_tensor(out=scbc[:, ci, 1:2], in0=gb[:, ci, :],
                                        in1=pbc[:, 1:2], op=OP.subtract)

            # ---- fp8 weights (gate q/qk production):
            # W8 = Wq * scale_c * 16 | Wc * 16  (Wc = Wp@Wv composed on host;
            # its GN input scale rides the xEn drain instead)
            w8 = cst.tile([128, 2, 2 * C], f8, tag="w8")
            for ci in range(2):
                nc.vector.tensor_scalar(
                    out=w8[:, ci, 0:C], in0=wall[:, ci, 0:C],
                    scalar1=scbc[:, ci, 0:1], scalar2=SW,
                    op0=OP.mult, op1=OP.mult)
                nc.vector.tensor_scalar(
                    out=w8[:, ci, C:2 * C], in0=wall[:, ci, C:2 * C],
                    scalar1=SW, scalar2=None, op0=OP.mult)

            qt = big.tile([128, 2, NH], f8, tag="qt")
            qk = big.tile([128, 2, NH], f8, tag="qk")

            def qt_unit(j):  # q for 512 queries: Wq' x (bias rides in qk)
                sl = slice(j * 512, (j + 1) * 512)
                pq = ps_s.tile([128, 2, 512], f32, tag="s", name=f"pq{j}")
                for cb in range(2):
                    nc.tensor.matmul(
                        pq[:, cb, :],
                        lhsT=w8[:, :, cb * 128:(cb + 1) * 128],
                        rhs=xf8[:, :, sl], start=True, stop=True,
                        perf_mode=PM.DoubleRow)
                nc.vector.tensor_scalar(
                    out=qt[:, :, sl], in0=pq, scalar1=1.0 / SW, scalar2=None,
                    op0=OP.mult)

            qt_unit(0)

            # ---- per-partition drain biases from the GN fold (bias_c on
            # unscaled bf16 W). The k-side bias shifts every logit of a query
            # equally -- softmax-invariant -- so it is dropped.
            bcr = gnp.tile([128, 2, 2], bf16, tag="bcr")  # bias_c, 2 copies
            for ci in range(2):
                nc.vector.tensor_copy(out=bcr[:, ci, 0:1],
                                      in_=scbc[:, ci, 1:2])
                nc.vector.tensor_copy(out=bcr[:, ci, 1:2],
                                      in_=scbc[:, ci, 1:2])
            # bias2[:, cb, 0] = qb + Wq @ bias_c ; bvp = Wc @ bias_c
            # (the latter is the composed v-path bias, pre-added into the
            # residual base on idle ACT below)
            bias2 = gnp.tile([128, 2, 1], f32, tag="bias2")
            bvp = gnp.tile([128, 2, 1], f32, tag="bvp")
            for wi, woff in ((0, 0), (1, C)):
                for cb in range(2):
                    pbias = ps_x.tile([128, 2], f32, tag="x")
                    for ci in range(2):
                        nc.tensor.matmul(
                            pbias,
                            lhsT=wall[:, ci, woff + cb * 128:
                                      woff + (cb + 1) * 128],
                            rhs=bcr[:, ci, :], start=(ci == 0), stop=(ci == 1))
                    if wi == 0:
                        nc.vector.tensor_tensor(
                            out=bias2[:, cb, 0:1], in0=pbias[:, 0:1],
                            in1=qb[:, cb, :], op=OP.add)
                    else:
                        nc.vector.tensor_copy(out=bvp[:, cb, :],
                                              in_=pbias[:, 0:1])
            xob = big.tile([128, 2, NH], f32, tag="xob")
            for co in range(2):
                nc.scalar.activation(out=xob[:, co, :], in_=xo[:, co, :],
                                     func=AF.Identity, bias=bvp[:, co, :])
            # fold the q bias into the qk drain instead of the q drain:
            # qk = s/16 * Wk16^T (q0 + bq) = (pk + qkb) * sc16,
            # qkb = Wk16^T bq16 / 16. qt then drains biasless in one op.
            bqr8 = gnp.tile([128, 2, 2], f8, tag="bqr8")
            for cb in range(2):
                for k in range(2):
                    nc.vector.tensor_scalar(
                        out=bqr8[:, cb, k:k + 1], in0=bias2[:, cb, 0:1],
                        scalar1=SW, scalar2=None, op0=OP.mult)
            qkbias = gnp.tile([128, 2, 1], f32, tag="qkbias")
            sc16 = gnp.tile([128, 2, 1], f32, tag="sc16")
            for ci in range(2):
                pqkb = ps_x.tile([128, 2], f32, tag="x")
                nc.tensor.matmul(
                    pqkb, lhsT=wkt8[:, :, ci * 128:(ci + 1) * 128],
                    rhs=bqr8, start=True, stop=True, perf_mode=PM.DoubleRow)
                nc.vector.tensor_scalar(
                    out=qkbias[:, ci, :], in0=pqkb[:, 0:1],
                    scalar1=1.0 / SW, scalar2=None, op0=OP.mult)
                nc.vector.tensor_scalar(
                    out=sc16[:, ci, :], in0=scbc[:, ci, 0:1],
                    scalar1=1.0 / SW, scalar2=None, op0=OP.mult)

            def qk_unit(j):  # qk = scale_c * (Wk^T q + Wk^T bq), 512 queries
                sl = slice(j * 512, (j + 1) * 512)
                pk = ps_s.tile([128, 2, 512], f32, tag="s", name=f"pqk{j}")
                for ci in range(2):
                    nc.tensor.matmul(
                        pk[:, ci, :],
                        lhsT=wkt8[:, :, ci * 128:(ci + 1) * 128],
                        rhs=qt[:, :, sl], start=True, stop=True,
                        perf_mode=PM.DoubleRow)
                for ci in range(2):
                    nc.vector.tensor_scalar(
                        out=qk[:, ci, sl], in0=pk[:, ci, :],
                        scalar1=qkbias[:, ci, :], scalar2=sc16[:, ci, :],
                        op0=OP.add, op1=OP.mult)

            state = {}

            def s_pair(j, p):
                sl = slice(j * 512, (j + 1) * 512)
                sp = ps_s.tile([128, 2, 512], f32, tag="s", name=f"sp{j}_{p}")
                for par in range(2):
                    mb = 2 * p + par
                    nc.tensor.matmul(
                        sp[:, par, :],
                        lhsT=xf8[:, :, mb * 128:(mb + 1) * 128],
                        rhs=qk[:, :, sl], start=True, stop=True,
                        perf_mode=PM.DoubleRow)
                return sp

            def xe_r(j, p, et):
                po, pr = state["po"], state["pr"]
                for ci in range(2):
                    nc.tensor.matmul(
                        po[:, ci, :],
                        lhsT=xt8[:, 2 * p:2 * p + 2, ci * 128:(ci + 1) * 128],
                        rhs=et, start=(p == 0), stop=(p == NP - 1),
                        perf_mode=PM.DoubleRow, skip_group_check=True)
                nc.tensor.matmul(
                    pr, lhsT=ones8, rhs=et,
                    start=(p == 0), stop=(p == NP - 1),
                    perf_mode=PM.DoubleRow, skip_group_check=True)

            def alloc_por(j):
                state["po"] = ps_o.tile([128, 2, 512], f32, tag="o",
                                        name=f"po{j}")
                state["pr"] = ps_r.tile([128, 512], f32, tag="r",
                                        name=f"pr{j}")

            def attention_all():
                """All 64 key-pair rounds as one flat software pipeline:
                the S pair for round g+1 is always in flight before round
                g's xE/R consumers, across chunk boundaries too (the et pool
                buffers the po/pr handoff)."""
                prev = None
                for g in range(4 * NP):
                    j, p = divmod(g, NP)
                    sp = s_pair(j, p)
                    if prev is not None:
                        jq, pq, etq = prev
                        if pq == 0 and jq >= 1:
                            tail_norm(jq - 1)
                            alloc_por(jq)
                        xe_r(jq, pq, etq)
                    if 2 <= p <= 3 and j >= 1:
                        tail_piece(j - 1, p - 2)
                    if p == 7 and j < 3:
                        qt_unit(j + 1)
                    if p == 11 and j < 3:
                        qk_unit(j + 1)
                    # logit shift keeps exp in e4m3 range (max logit ~8:
                    # e^(8-3)=148 < 240); softmax is shift-invariant
                    et = epool.tile([128, 2, 512], f8, tag="et")
                    nc.scalar.activation(out=et, in_=sp, func=AF.Exp,
                                         scale=1.0 / SW, bias=negc)
                    prev = (j, p, et)
                xe_r(3, NP - 1, prev[2])

            def tail_norm(j):
                """Free po/pr: normalized xE in fp8, 1/R via fast recip."""
                po, pr = state["po"], state["pr"]
                rb = wrk.tile([128, 512], f32, tag="rb")
                nc.vector.reciprocal_approx_fast(out=rb, in_=pr)  # frees pr
                xen = wrk.tile([128, 2, 512], f8, tag="xen")
                onorm = wrk.tile([128, 2, 512], f32, tag="onorm")
                nc.vector.tensor_copy(out=onorm, in_=po)  # frees po
                for ci in range(2):
                    nc.vector.scalar_tensor_tensor(
                        out=xen[:, ci, :], in0=onorm[:, ci, :],
                        scalar=scbc[:, ci, 0:1], in1=rb,
                        op0=OP.mult, op1=OP.mult)
                state[f"xen{j}"] = xen

            def final_tail(j):
                """Last chunk's tail: nothing overlaps it, so pipeline it in
                query halves through the now-free ps_s banks."""
                po, pr = state["po"], state["pr"]
                sl0 = j * 512
                for h in range(2):
                    q = slice(h * 256, (h + 1) * 256)
                    rb = wrk.tile([128, 256], f32, tag="rbh", name=f"rb{h}")
                    nc.vector.reciprocal_approx_fast(out=rb, in_=pr[:, q])
                    xen = wrk.tile([128, 2, 256], f8, tag="xenh",
                                   name=f"xen{h}")
                    for ci in range(2):
                        nc.vector.scalar_tensor_tensor(
                            out=xen[:, ci, :], in0=po[:, ci, q],
                            scalar=scbc[:, ci, 0:1], in1=rb,
                            op0=OP.mult, op1=OP.mult)
                    for co in range(2):
                        pp = ps_s.tile([128, 2, 512], f32, tag="s",
                                       name=f"fpp{h}_{co}")
                        nc.tensor.matmul(
                            pp[:, 0, 0:256],
                            lhsT=w8[:, :, C + co * 128:C + (co + 1) * 128],
                            rhs=xen, start=True, stop=True,
                            perf_mode=PM.DoubleRow)
                        outt = wrk.tile([128, 256], f32, tag="outth",
                                        name=f"outt{h}_{co}")
                        nc.vector.scalar_tensor_tensor(
                            out=outt, in0=pp[:, 0, 0:256], scalar=1.0 / SW,
                            in1=xob[:, co, sl0 + h * 256:sl0 + (h + 1) * 256],
                            op0=OP.mult, op1=OP.add)
                        eng = nc.sync if co == 0 else nc.scalar
                        eng.dma_start(
                            out=d_out.ap()[:, co,
                                           sl0 + h * 256:sl0 + (h + 1) * 256],
                            in_=outt)

            def tail_piece(j, step):
                """One composed-matmul piece: out = Wc (s xEn) / 16 + xob."""
                sl = slice(j * 512, (j + 1) * 512)
                co = step
                xen = state[f"xen{j}"]
                pp = ps_x.tile([128, 512], f32, tag="x", name=f"pp{j}_{co}")
                nc.tensor.matmul(
                    pp, lhsT=w8[:, :, C + co * 128:C + (co + 1) * 128],
                    rhs=xen, start=True, stop=True,
                    perf_mode=PM.DoubleRow)
                outt = wrk.tile([128, 512], f32, tag="outt")
                nc.vector.scalar_tensor_tensor(
                    out=outt, in0=pp, scalar=1.0 / SW,
                    in1=xob[:, co, sl], op0=OP.mult, op1=OP.add)
                nc.sync.dma_start(out=d_out.ap()[:, co, sl], in_=outt)
                if co == 1:
                    state.pop(f"xen{j}")

            # ---- one flat attention pipeline over all 64 pair rounds ----
            qk_unit(0)
            alloc_por(0)
            attention_all()
            final_tail(3)

    nc.compile()
    _cache["nc"] = nc
    return nc


def _prep_maps(x, gn_w, gn_b, qkv_w, qkv_b, proj_w, proj_b):
    """Host-side sharding + layout prep. Returns list of 8 in_maps."""
    import ml_dtypes
    f8 = ml_dtypes.float8_e4m3
    bf16 = ml_dtypes.bfloat16

    x = np.asarray(x, np.float32)
    qkv_w = np.asarray(qkv_w, np.float32)
    qkv_b = np.asarray(qkv_b, np.float32)
    proj_w = np.asarray(proj_w, np.float32)
    proj_b = np.asarray(proj_b, np.float32)
    gn_w = np.asarray(gn_w, np.float32)
    gn_b = np.asarray(gn_b, np.float32)

    def chunked(a):  # [256, ...] -> [128, 2, ...]
        return np.ascontiguousarray(a.reshape(2, 128, *a.shape[1:]).transpose(
            1, 0, *range(2, a.ndim + 1)))

    wq = qkv_w[0:C].T
    wc = (proj_w @ qkv_w[2 * C:3 * C]).T    # composed v->out weight
    wall = chunked(np.concatenate([wq, wc], axis=1)).astype(bf16)
    # wkt: Wk as [c_out, c_in], raw (no GN scale), pre-scaled by 16 for fp8
    wkt = chunked(qkv_w[C:2 * C] * SW).astype(f8)
    rbias = proj_w @ qkv_b[2 * C:3 * C] + proj_b   # v-bias fold + proj bias
    smalls = chunked(np.stack([qkv_b[0:C], qkv_b[C:2 * C], gn_b], axis=1))

    cidx = np.arange(C)
    ag_full = (cidx[:, None] // CPG == np.arange(G)[None, :]).astype(np.float32)
    ag = chunked(ag_full / CPG)                     # [128, 2, G], carries 1/8
    bg_full = ag_full * gn_w[:, None]               # carries gn_w
    bg = np.ascontiguousarray(
        bg_full.reshape(2, 128, G).transpose(2, 0, 1))  # [G, 2, 128]

    maps = []
    for core in range(8):
        b, half = core // 2, core % 2
        xflat = x[b].reshape(C, HW)
        own = xflat[:, half * NH:(half + 1) * NH]
        other = xflat[:, (1 - half) * NH:(2 - half) * NH]
        xroll = np.concatenate([own, other], axis=1)  # own queries first
        xt = np.ascontiguousarray(
            xroll.T.reshape(MB, 128, C).transpose(1, 0, 2))
        xo = own + rbias[:, None]
        maps.append({
            "xf": chunked(xroll).astype(f8),
            "xt": xt.astype(f8),
            "xo": chunked(xo),
            "w": wall, "wkt": wkt, "sb": smalls, "ag": ag, "bg": bg,
        })
    return maps


def kernel(x, gn_w, gn_b, qkv_w, qkv_b, proj_w, proj_b):
    import concourse.bass_utils as bu
    nc = build_nc()
    maps = _prep_maps(x, gn_w, gn_b, qkv_w, qkv_b, proj_w, proj_b)
    res = bu.run_bass_kernel_spmd(nc, maps, core_ids=list(range(8)))
    out = np.empty((B, C, HW), np.float32)
    for core in range(8):
        b, half = core // 2, core % 2
        o = res.results[core]["out"]                # [128, 2, NH]
        out[b, :, half * NH:(half + 1) * NH] = \
            o.transpose(1, 0, 2).reshape(C, NH)
    return out.reshape(B, C, 64, 64)
